# revision 1
# baseline (speedup 1.0000x reference)
"""Trainium2 Bass kernel for nn_CrossAttentionModel (8 NeuronCores).

Strategy: the only large tensors are the 4x4-downsampled activations
(a: 128x3x20480, v: 128x20480, 42 MB) and the encoder weights
W1/W2 (128x20480 each, 21 MB).  We shard the CONTRACTION dim d=20480
across the 8 cores (d-parallel): each core streams its 2560-row slice
of both the activations (all 128 samples) and the weights, accumulates
partial encoder outputs aud/vis for every sample, and a ReduceScatter
(sum over d-shards, scatter over samples) hands each core the exact
encoder outputs for its 16 samples.  This reads every weight byte and
every activation byte exactly once -> minimal HBM traffic
(63 MB / 8 cores ~ 7.9 MB/core).  The small attention head then runs
batch-parallel (16 samples/core) on-chip.

Host-side prep only does the nearest-downsample slicing + layout
transposes (sharding); all FLOPs run on device.
"""
import sys
sys.path.insert(0, "/opt/trn_rl_repo")

import numpy as np
import concourse.bass as bass
import concourse.mybir as mybir
import concourse.tile as tile
from concourse import bacc
from concourse.bass_utils import run_bass_kernel_spmd

F32 = mybir.dt.float32

# ---- problem constants (hardcoded; kernel.py must be self-contained) ----
B, C, H, W = 128, 3, 512, 640
D = 20480            # (H//4) * (W//4)
DE = 128             # encoder dim
DA = 32              # attention dim
NC_ = 8              # cores
DL = D // NC_        # 2560 d-rows per core
NT = DL // 128       # 20 k-tiles per core
SL = B // NC_        # 16 samples per core (post reduce-scatter)
NQ = 4               # sample quads per core (chunks of 512 = 4*128 cols)

# encoder matmul dtype: float32r = fp32 bits, fast PE path
EDT = mybir.dt.float32r
# attention compute dtype
CDT = mybir.dt.bfloat16


def _np_dt(dt):
    return mybir.dt.np(dt)


def build_bass():
    nc = bacc.Bacc("TRN2", target_bir_lowering=False, debug=False,
                   num_devices=NC_)

    # ---- per-core DRAM parameters (shards supplied host-side) ----
    aT = nc.declare_dram_parameter("aT", [DL, 3 * B], EDT, isOutput=False)
    vT = nc.declare_dram_parameter("vT", [DL, B], EDT, isOutput=False)
    w12 = nc.declare_dram_parameter("w12", [DL, 2 * DE], EDT, isOutput=False)
    brow = nc.declare_dram_parameter("brow", [1, 2 * DE], EDT, isOutput=False)
    ones = nc.declare_dram_parameter("ones", [1, 3 * B], EDT, isOutput=False)
    aaT3 = nc.declare_dram_parameter("aaT3", [3, 3], CDT, isOutput=False)
    avT3 = nc.declare_dram_parameter("avT3", [3, 3], CDT, isOutput=False)
    wa3 = nc.declare_dram_parameter("wa3", [3, DA], CDT, isOutput=False)
    wv3 = nc.declare_dram_parameter("wv3", [3, DA], CDT, isOutput=False)
    wcaT = nc.declare_dram_parameter("wcaT", [2 * DE, DA], CDT, isOutput=False)
    wcvT = nc.declare_dram_parameter("wcvT", [2 * DE, DA], CDT, isOutput=False)
    whaT = nc.declare_dram_parameter("whaT", [DA, 3], CDT, isOutput=False)
    whvT = nc.declare_dram_parameter("whvT", [DA, 3], CDT, isOutput=False)
    out = nc.declare_dram_parameter("out", [SL, 3, 2 * DE], F32, isOutput=True)

    # internal DRAM for the collective
    P = nc.dram_tensor("P", [B, 4, DE], F32)
    P_rs = nc.dram_tensor("P_rs", [SL, 4, DE], F32)

    SK = SL * DE  # 2048 free columns in (sample, enc) layout

    with tile.TileContext(nc) as tc:
        with (
            tc.tile_pool(name="consts", bufs=1) as cpool,
            tc.tile_pool(name="enc_in", bufs=3) as epool,
            tc.tile_pool(name="sb", bufs=1) as sb,
        ):
            # ---------- small consts ----------
            brow_t = cpool.tile([1, 2 * DE], EDT)
            nc.gpsimd.dma_start(brow_t[:], brow[:])
            ones_t = cpool.tile([1, 3 * B], EDT)
            nc.gpsimd.dma_start(ones_t[:], ones[:])
            aa_t = cpool.tile([3, 3], CDT)
            nc.gpsimd.dma_start(aa_t[:], aaT3[:])
            av_t = cpool.tile([3, 3], CDT)
            nc.gpsimd.dma_start(av_t[:], avT3[:])
            wa3_t = cpool.tile([3, DA], CDT)
            nc.gpsimd.dma_start(wa3_t[:], wa3[:])
            wv3_t = cpool.tile([3, DA], CDT)
            nc.gpsimd.dma_start(wv3_t[:], wv3[:])
            wca_lo = cpool.tile([DE, DA], CDT)
            nc.gpsimd.dma_start(wca_lo[:], wcaT[0:DE, :])
            wca_hi = cpool.tile([DE, DA], CDT)
            nc.gpsimd.dma_start(wca_hi[:], wcaT[DE:2 * DE, :])
            wcv_lo = cpool.tile([DE, DA], CDT)
            nc.gpsimd.dma_start(wcv_lo[:], wcvT[0:DE, :])
            wcv_hi = cpool.tile([DE, DA], CDT)
            nc.gpsimd.dma_start(wcv_hi[:], wcvT[DE:2 * DE, :])
            wha_t = cpool.tile([DA, 3], CDT)
            nc.gpsimd.dma_start(wha_t[:], whaT[:])
            whv_t = cpool.tile([DA, 3], CDT)
            nc.gpsimd.dma_start(whv_t[:], whvT[:])

            # ---------- phase 1: encoder (d-sharded, all 128 samples) ----------
            # psum_c[s, 0:DE] accumulates (a_c @ W1^T)[s, :] for channel c;
            # psum_v[s, DE:2DE] accumulates (v @ W2^T)[s, :].
            with tc.tile_pool(name="enc_ps", bufs=1, space="PSUM") as eps:
                psums = [eps.tile([B, 2 * DE], F32, tag=f"enc{c}",
                                  name=f"psum_enc{c}") for c in range(4)]
                # bias init (start=True clears the bank): out[s, k] = b/8
                for c in range(3):
                    nc.tensor.matmul(psums[c][:], ones_t[0:1, 0:B],
                                     brow_t[:], start=True, stop=False)
                nc.tensor.matmul(psums[3][:], ones_t[0:1, 0:B],
                                 brow_t[:], start=True, stop=False)

                for t in range(NT):
                    r0 = t * 128
                    at3 = epool.tile([128, 3 * B], EDT, tag="at3")
                    nc.gpsimd.dma_start(at3[:], aT[r0:r0 + 128, :])
                    vt = epool.tile([128, B], EDT, tag="vt")
                    nc.gpsimd.dma_start(vt[:], vT[r0:r0 + 128, :])
                    w12t = epool.tile([128, 2 * DE], EDT, tag="w12t")
                    nc.gpsimd.dma_start(w12t[:], w12[r0:r0 + 128, :])
                    last = t == NT - 1
                    for c in range(3):
                        nc.tensor.matmul(psums[c][:],
                                         at3[:, c * B:(c + 1) * B], w12t[:],
                                         start=False, stop=last)
                    nc.tensor.matmul(psums[3][:], vt[:], w12t[:],
                                     start=False, stop=last)

                # evict to P: channels use cols 0:DE (W1 half), vis DE:2DE
                for c in range(3):
                    ev = sb.tile([B, DE], F32, tag="ev")
                    nc.any.tensor_copy(ev[:], psums[c][:, 0:DE])
                    nc.gpsimd.dma_start(P[:, c, :], ev[:])
                ev = sb.tile([B, DE], F32, tag="ev")
                nc.any.tensor_copy(ev[:], psums[3][:, DE:2 * DE])
                nc.gpsimd.dma_start(P[:, 3, :], ev[:])

            # ---------- reduce-scatter: sum over d-shards, scatter samples ----
            nc.gpsimd.collective_compute(
                "ReduceScatter", mybir.AluOpType.add,
                replica_groups=[list(range(NC_))],
                ins=[P[:]], outs=[P_rs[:]],
            )

            # ---------- load this core's 16 samples: [ch, (s, k)] layout ----
            # aud channels and vis kept in separate partition-0-based tiles
            # (compute engines are lane-locked; no partition shifts allowed)
            av_a = sb.tile([3, SK], F32, tag="av_a")   # aud: enc1 + b1
            av_v = sb.tile([3, SK], F32, tag="av_v")   # vis (3 equal rows)
            nc.gpsimd.dma_start(
                av_a[:].rearrange("c (s k) -> c s k", k=DE),
                P_rs[:, 0:3, :].transpose([1, 0, 2]))
            for r in range(3):
                nc.gpsimd.dma_start(
                    av_v[r:r + 1, :].rearrange("c (s k) -> c s k", k=DE),
                    P_rs[:, 3:4, :].transpose([1, 0, 2]))
            av16_a = sb.tile([3, SK], CDT, tag="av16_a")
            nc.any.tensor_copy(av16_a[:], av_a[:])
            av16_v = sb.tile([3, SK], CDT, tag="av16_v")
            nc.any.tensor_copy(av16_v[:], av_v[:])

            with (
                tc.tile_pool(name="att_ps", bufs=2, space="PSUM") as aps,
                tc.tile_pool(name="h_ps", bufs=1, space="PSUM") as hps,
                tc.tile_pool(name="b_ps", bufs=1, space="PSUM") as bps,
                tc.tile_pool(name="o_ps", bufs=1, space="PSUM") as ops_,
            ):
                # ---------- B = A @ av: four K=3 products [3, SK] ----------
                # (Aa|Av) x (aud-half | vis-half) of av
                ba_lo = sb.tile([3, SK], CDT, tag="ba_lo")
                ba_hi = sb.tile([3, SK], CDT, tag="ba_hi")
                bv_lo = sb.tile([3, SK], CDT, tag="bv_lo")
                bv_hi = sb.tile([3, SK], CDT, tag="bv_hi")
                bspec = [(ba_lo, aa_t, av16_a), (ba_hi, aa_t, av16_v),
                         (bv_lo, av_t, av16_a), (bv_hi, av_t, av16_v)]
                for q in range(NQ):
                    ck = slice(q * 512, (q + 1) * 512)
                    for dst, lhs_c, rhs_c in bspec:
                        pb = bps.tile([3, 512], F32, tag="pb")
                        nc.tensor.matmul(pb[:], lhs_c[:], rhs_c[:, ck],
                                         start=True, stop=True)
                        nc.any.tensor_copy(dst[:, ck], pb[:])

                # ---------- attention maps: att = tanh((enc^T @ B) / 16) -------
                att = {
                    (br, half): sb.tile([DE, SK], CDT, tag=f"att_{br}_{half}",
                                        name=f"att_{br}_{half}")
                    for br in ("a", "v") for half in (0, 1)
                }
                blos = {"a": (ba_lo, ba_hi), "v": (bv_lo, bv_hi)}
                enc_rhs = {"a": av16_a, "v": av16_v}
                for q in range(NQ):
                    for br in ("a", "v"):
                        rhs_t = enc_rhs[br]
                        for half in (0, 1):
                            blk = blos[br][half]
                            pa = aps.tile([DE, 512], F32, tag="attp")
                            for j in range(4):
                                s = q * 4 + j
                                sl_ = slice(s * DE, (s + 1) * DE)
                                nc.tensor.matmul(
                                    pa[:, j * DE:(j + 1) * DE],
                                    blk[:, sl_], rhs_t[:, sl_],
                                    start=True, stop=True)
                            nc.scalar.activation(
                                att[(br, half)][:, q * 512:(q + 1) * 512], pa[:],
                                mybir.ActivationFunctionType.Tanh, scale=0.0625)

                # ---------- H = relu(att @ Wc^T + enc^T @ W) ----------
                ht_a = sb.tile([DA, SK], CDT, tag="ht_a")
                ht_v = sb.tile([DA, SK], CDT, tag="ht_v")
                for q in range(NQ):
                    ck = slice(q * 512, (q + 1) * 512)
                    ph_a = hps.tile([DA, 512], F32, tag="ph_a")
                    nc.tensor.matmul(ph_a[:], wa3_t[:], av16_a[:, ck],
                                     start=True, stop=False)
                    nc.tensor.matmul(ph_a[:], wca_lo[:], att[("a", 0)][:, ck],
                                     start=False, stop=False)
                    nc.tensor.matmul(ph_a[:], wca_hi[:], att[("a", 1)][:, ck],
                                     start=False, stop=True)
                    nc.scalar.activation(ht_a[:, ck], ph_a[:],
                                         mybir.ActivationFunctionType.Relu)
                    ph_v = hps.tile([DA, 512], F32, tag="ph_v")
                    nc.tensor.matmul(ph_v[:], wv3_t[:], av16_v[:, ck],
                                     start=True, stop=False)
                    nc.tensor.matmul(ph_v[:], wcv_lo[:], att[("v", 0)][:, ck],
                                     start=False, stop=False)
                    nc.tensor.matmul(ph_v[:], wcv_hi[:], att[("v", 1)][:, ck],
                                     start=False, stop=True)
                    nc.scalar.activation(ht_v[:, ck], ph_v[:],
                                         mybir.ActivationFunctionType.Relu)

                # ---------- out = Wh @ H^T + enc ----------
                outa = sb.tile([3, SK], F32, tag="outa")
                outv = sb.tile([3, SK], F32, tag="outv")
                for q in range(NQ):
                    ck = slice(q * 512, (q + 1) * 512)
                    poa = ops_.tile([3, 512], F32, tag="poa")
                    nc.tensor.matmul(poa[:], wha_t[:], ht_a[:, ck],
                                     start=True, stop=True)
                    nc.vector.tensor_add(outa[:, ck], poa[:], av_a[:, ck])
                    pov = ops_.tile([3, 512], F32, tag="pov")
                    nc.tensor.matmul(pov[:], whv_t[:], ht_v[:, ck],
                                     start=True, stop=True)
                    nc.vector.tensor_add(outv[:, ck], pov[:], av_v[:, ck])

            nc.gpsimd.dma_start(
                out[:, :, 0:DE].transpose([1, 0, 2]),
                outa[:].rearrange("c (s k) -> c s k", k=DE))
            nc.gpsimd.dma_start(
                out[:, :, DE:2 * DE].transpose([1, 0, 2]),
                outv[:].rearrange("c (s k) -> c s k", k=DE))

    nc.compile()
    return nc


_NC_CACHE = None


def _get_nc():
    global _NC_CACHE
    if _NC_CACHE is None:
        _NC_CACHE = build_bass()
    return _NC_CACHE


def _prep_inputs(f1_norm, f2_norm, W1, b1, W2, b2, Aa, Av, Wa, Wv,
                 Wca, Wcv, Wha, Whv):
    f1_norm = np.asarray(f1_norm, dtype=np.float32)
    f2_norm = np.asarray(f2_norm, dtype=np.float32)
    edt = _np_dt(EDT)
    cdt = _np_dt(CDT)

    # nearest-downsample + transpose to [d, (c, s)] / [d, s]
    a_ds = f1_norm[:, :, ::4, ::4].reshape(B, 3, D)       # (B, 3, D)
    aT_full = np.ascontiguousarray(a_ds.transpose(2, 1, 0)
                                   .reshape(D, 3 * B)).astype(edt, copy=False)
    v_ds = f2_norm[:, ::4, ::4].reshape(B, D)
    vT_full = np.ascontiguousarray(v_ds.T).astype(edt, copy=False)
    w12_full = np.ascontiguousarray(
        np.concatenate([np.asarray(W1).T, np.asarray(W2).T], axis=1)
    ).astype(edt, copy=False)                              # (D, 256)

    brow = np.concatenate([np.asarray(b1), np.asarray(b2)])[None, :] / NC_
    brow = brow.astype(edt)
    ones = np.ones((1, 3 * B), dtype=edt)

    consts = {
        "brow": brow, "ones": ones,
        "aaT3": np.ascontiguousarray(np.asarray(Aa).T).astype(cdt),
        "avT3": np.ascontiguousarray(np.asarray(Av).T).astype(cdt),
        "wa3": np.ascontiguousarray(np.asarray(Wa).T).astype(cdt),
        "wv3": np.ascontiguousarray(np.asarray(Wv).T).astype(cdt),
        "wcaT": np.ascontiguousarray(np.asarray(Wca).T).astype(cdt),
        "wcvT": np.ascontiguousarray(np.asarray(Wcv).T).astype(cdt),
        "whaT": np.ascontiguousarray(np.asarray(Wha).T).astype(cdt),
        "whvT": np.ascontiguousarray(np.asarray(Whv).T).astype(cdt),
    }

    in_maps = []
    for i in range(NC_):
        rs = slice(i * DL, (i + 1) * DL)
        m = {
            "aT": np.ascontiguousarray(aT_full[rs]),
            "vT": np.ascontiguousarray(vT_full[rs]),
            "w12": np.ascontiguousarray(w12_full[rs]),
        }
        m.update(consts)
        in_maps.append(m)
    return in_maps


def _run(inputs, trace=False):
    nc = _get_nc()
    in_maps = _prep_inputs(**inputs)
    res = run_bass_kernel_spmd(nc, in_maps, list(range(NC_)), trace=trace)
    full = np.concatenate([res.results[i]["out"] for i in range(NC_)], axis=0)
    return full.astype(np.float32, copy=False), res


def kernel(**inputs):
    out, _ = _run(inputs, trace=False)
    return out



# revision 16
# speedup vs baseline: 1.1813x; 1.1813x over previous
"""Trainium2 Bass kernel for nn_CrossAttentionModel (8 NeuronCores).

Strategy (v2): d-parallel encoder + batch-parallel attention, everything
bf16 on the wire.

Encoder: the contraction dim d=20480 is sharded 8 ways.  Each core
streams a host-packed [20, 128, 768] bf16 blob (aT | vT | w1T | w2T per
128-row d-tile) with ONE DMA per tile, and runs weight-stationary
matmuls into a [enc=128, (c,s)=512] psum accumulator (2 LDW + 2 MM per
tile).  The psum is PE-transposed to sample-major, cast to bf16, and
ReduceScattered (sum over d-shards, scatter over samples) so each core
ends with exact encoder outputs for its 16 samples.  Optionally the
d range is split in two so the first ReduceScatter overlaps the second
half of the encoder stream.

Attention: per-core 16 samples in a [(s,i)=48, .] partition layout.
The 3x3 channel mixes (Aa/Av) are ONE matmul each via host-built
block-diagonal kron(I16, A^T) stationaries.  The per-sample K=3
attention maps accumulate into [128, (s,k)] psum chunks, tanh on the
ACT engine, then the H/out projections are dense K=128/K=32 matmuls.
"""
import sys
sys.path.insert(0, "/opt/trn_rl_repo")

import numpy as np
import concourse.bass as bass
import concourse.mybir as mybir
import concourse.tile as tile
from concourse import bacc
from concourse.bass_utils import run_bass_kernel_spmd

F32 = mybir.dt.float32
BF16 = mybir.dt.bfloat16
AF = mybir.ActivationFunctionType

# ---- problem constants (hardcoded; kernel.py must be self-contained) ----
B, C, H, W = 128, 3, 512, 640
D = 20480            # (H//4) * (W//4)
DE = 128             # encoder dim
DA = 32              # attention dim
NC_ = 8              # cores
DL = D // NC_        # 2560 d-rows per core
NT = DL // 128       # 20 k-tiles per core
SL = B // NC_        # 16 samples per core (post reduce-scatter)
SK = SL * DE         # 2048 = (sample, enc) flattened cols per core

# d-split points for the encoder psum accumulation.  Collectives on this
# fabric are latency-bound (~8-10us regardless of payload, measured) and
# serialize on the cc stream, so a single ReduceScatter beats a pipelined
# pair.
HALVES = [(0, 20)]


def _np_dt(dt):
    return mybir.dt.np(dt)


def build_bass():
    nc = bacc.Bacc("TRN2", target_bir_lowering=False, debug=False,
                   num_devices=NC_)

    # ---- per-core DRAM parameters ----
    pack = nc.declare_dram_parameter("pack", [NT, 128, 768], BF16,
                                     isOutput=False)
    ident = nc.declare_dram_parameter("ident", [128, 128], BF16,
                                      isOutput=False)
    ones384 = nc.declare_dram_parameter("ones384", [1, 384], BF16,
                                        isOutput=False)
    b1col = nc.declare_dram_parameter("b1col", [1, 128], BF16, isOutput=False)
    b2col = nc.declare_dram_parameter("b2col", [1, 128], BF16, isOutput=False)
    bdAaT = nc.declare_dram_parameter("bdAaT", [48, 48], BF16, isOutput=False)
    bdAvT = nc.declare_dram_parameter("bdAvT", [48, 48], BF16, isOutput=False)
    wa3 = nc.declare_dram_parameter("wa3", [3, DA], BF16, isOutput=False)
    wv3 = nc.declare_dram_parameter("wv3", [3, DA], BF16, isOutput=False)
    wcaT = nc.declare_dram_parameter("wcaT", [2 * DE, DA], BF16,
                                     isOutput=False)
    wcvT = nc.declare_dram_parameter("wcvT", [2 * DE, DA], BF16,
                                     isOutput=False)
    wh6 = nc.declare_dram_parameter("wh6", [2 * DA, 6], BF16, isOutput=False)
    out = nc.declare_dram_parameter("out", [SL, 3, 2 * DE], F32,
                                    isOutput=True)
    DBG = False
    if DBG:
        dbg_av48 = nc.declare_dram_parameter("dbg_av48", [48, 256], F32,
                                             isOutput=True)
        dbg_b48 = nc.declare_dram_parameter("dbg_b48", [48, 512], F32,
                                            isOutput=True)
        dbg_bda = nc.declare_dram_parameter("dbg_bda", [48, SK], F32,
                                            isOutput=True)
        dbg_atta0 = nc.declare_dram_parameter("dbg_atta0", [128, SK], F32,
                                              isOutput=True)
        dbg_ht = nc.declare_dram_parameter("dbg_ht", [2 * DA, SK], F32,
                                           isOutput=True)

    # internal DRAM for the collectives
    n_h = len(HALVES)
    P_h = [nc.dram_tensor(f"P_{h}", [B, 4, DE], BF16) for h in range(n_h)]
    P_rs = [nc.dram_tensor(f"P_rs_{h}", [SL, 4, DE], BF16)
            for h in range(n_h)]

    grp = [list(range(NC_))]

    with tile.TileContext(nc) as tc:
        with (
            tc.tile_pool(name="consts", bufs=1) as cpool,
            tc.tile_pool(name="stream", bufs=3) as spool,
            tc.tile_pool(name="sb", bufs=1) as sb,
        ):
            # ---------- small consts ----------
            ident_t = cpool.tile([128, 128], BF16)
            nc.gpsimd.dma_start(ident_t[:], ident[:])
            ones_t = cpool.tile([1, 384], BF16)
            nc.gpsimd.dma_start(ones_t[:], ones384[:])
            b1_t = cpool.tile([1, 128], BF16)
            nc.gpsimd.dma_start(b1_t[:], b1col[:])
            b2_t = cpool.tile([1, 128], BF16)
            nc.gpsimd.dma_start(b2_t[:], b2col[:])
            bdAa_t = cpool.tile([48, 48], BF16)
            nc.gpsimd.dma_start(bdAa_t[:], bdAaT[:])
            bdAv_t = cpool.tile([48, 48], BF16)
            nc.gpsimd.dma_start(bdAv_t[:], bdAvT[:])
            wa3_t = cpool.tile([3, DA], BF16)
            nc.gpsimd.dma_start(wa3_t[:], wa3[:])
            wv3_t = cpool.tile([3, DA], BF16)
            nc.gpsimd.dma_start(wv3_t[:], wv3[:])
            wca_lo = cpool.tile([DE, DA], BF16)
            nc.gpsimd.dma_start(wca_lo[:], wcaT[0:DE, :])
            wca_hi = cpool.tile([DE, DA], BF16)
            nc.gpsimd.dma_start(wca_hi[:], wcaT[DE:2 * DE, :])
            wcv_lo = cpool.tile([DE, DA], BF16)
            nc.gpsimd.dma_start(wcv_lo[:], wcvT[0:DE, :])
            wcv_hi = cpool.tile([DE, DA], BF16)
            nc.gpsimd.dma_start(wcv_hi[:], wcvT[DE:2 * DE, :])
            wh6_t = cpool.tile([2 * DA, 6], BF16)
            nc.gpsimd.dma_start(wh6_t[:], wh6[:])

            # block-diag enc tiles, zeroed early (runs during the encoder)
            bd_a = sb.tile([48, SK], BF16, tag="bd_a")
            nc.vector.memset(bd_a[:], 0.0)
            bd_v = sb.tile([48, SK], BF16, tag="bd_v")
            nc.gpsimd.memset(bd_v[:], 0.0)

            # ---------- phase 1: encoder (d-sharded, all 128 samples) ----
            # psum[k_enc, col]: cols 0:384 = a@W1 partials for (c,s),
            # cols 384:512 = v@W2 partials.  One psum per d-half.
            with (
                tc.tile_pool(name="enc_ps", bufs=1, space="PSUM") as eps,
                tc.tile_pool(name="tr_ps", bufs=2, space="PSUM") as tps,
            ):
                psums = [eps.tile([128, 512], F32, tag=f"enc{h}",
                                  name=f"psum_enc{h}")
                         for h in range(n_h)]

                def evict_half(h):
                    # cast to bf16, PE-transpose each 128-block to
                    # sample-major, cast again, DMA to P_h
                    ev = sb.tile([128, 512], BF16, tag=f"ev{h}")
                    nc.vector.tensor_copy(ev[:], psums[h][:])
                    ptr = tps.tile([128, 512], BF16, tag="ptr")
                    for c in range(4):
                        cb = slice(c * 128, (c + 1) * 128)
                        nc.tensor.transpose(ptr[:, cb], ev[:, cb], ident_t[:])
                    evt = sb.tile([128, 512], BF16, tag=f"evt{h}")
                    nc.vector.tensor_copy(evt[:], ptr[:])
                    nc.gpsimd.dma_start(
                        P_h[h][:].rearrange("s c k -> s (c k)"), evt[:])
                    nc.gpsimd.collective_compute(
                        "ReduceScatter", mybir.AluOpType.add,
                        replica_groups=grp,
                        ins=[P_h[h][:]], outs=[P_rs[h][:]],
                    )

                for h, (t0, t1) in enumerate(HALVES):
                    for t in range(t0, t1):
                        tl = spool.tile([128, 768], BF16, tag="tl")
                        nc.sync.dma_start(tl[:], pack[t, :, :])
                        first, last = t == t0, t == t1 - 1
                        if h == 0 and first:
                            # bias/NC_ seeded once (summed by the RS)
                            nc.tensor.matmul(psums[h][:, 0:384], b1_t[:],
                                             ones_t[:, 0:384],
                                             start=True, stop=False)
                            nc.tensor.matmul(psums[h][:, 384:512], b2_t[:],
                                             ones_t[:, 0:128],
                                             start=True, stop=False)
                            first = False
                        nc.tensor.matmul(psums[h][:, 0:384],
                                         tl[:, 512:640], tl[:, 0:384],
                                         start=first, stop=last)
                        nc.tensor.matmul(psums[h][:, 384:512],
                                         tl[:, 640:768], tl[:, 384:512],
                                         start=first, stop=last)
                        # emit the previous half's eviction a few tiles in
                        # so its DVE cast is ready and PE doesn't stall
                        if h == 1 and t == t0 + 2:
                            evict_half(0)
                evict_half(n_h - 1)

            # ---------- phase 2: attention (batch-parallel, 16 samples) ---
            # av48[(s,i), 0:128]=aud, [128:256]=vis ; av6a/av6v [3, (s,k)]
            av48_h = []
            av6a_h = []
            av6v_h = []
            for h in range(n_h):
                avh = sb.tile([48, 256], BF16, tag=f"av48_{h}")
                a3 = avh[:, 0:128].rearrange("(s i) k -> s i k", i=3)
                v3 = avh[:, 128:256].rearrange("(s i) k -> s i k", i=3)
                for i in range(3):
                    nc.gpsimd.dma_start(a3[:, i, :], P_rs[h][:, i, :])
                    nc.gpsimd.dma_start(v3[:, i, :], P_rs[h][:, 3, :])
                av48_h.append(avh)
                a6 = sb.tile([3, SK], BF16, tag=f"av6a_{h}")
                nc.gpsimd.dma_start(
                    a6[:].rearrange("c (s k) -> c s k", k=DE),
                    P_rs[h][:, 0:3, :].transpose([1, 0, 2]))
                av6a_h.append(a6)
                v6 = sb.tile([3, SK], BF16, tag=f"av6v_{h}")
                for c in range(3):
                    nc.gpsimd.dma_start(
                        v6[c:c + 1, :].rearrange("o (s k) -> o s k", k=DE),
                        P_rs[h][:, 3:4, :].transpose([1, 0, 2]))
                av6v_h.append(v6)

            if n_h == 1:
                av48, av6a, av6v = av48_h[0], av6a_h[0], av6v_h[0]
            else:
                av48 = sb.tile([48, 256], BF16, tag="av48")
                av6a = sb.tile([3, SK], BF16, tag="av6a")
                av6v = sb.tile([3, SK], BF16, tag="av6v")
                with nc.allow_low_precision(reason="bf16 partial merge"):
                    nc.vector.tensor_add(av48[:], av48_h[0][:], av48_h[1][:])
                    nc.vector.tensor_add(av6a[:], av6a_h[0][:], av6a_h[1][:])
                    nc.vector.tensor_add(av6v[:], av6v_h[0][:], av6v_h[1][:])

            # combined [aud; vis] rows for the residual add (lane-aligned
            # with the packed out-proj psum)
            av6 = sb.tile([6, SK], BF16, tag="av6")
            nc.scalar.dma_start(
                av6[0:3, :].rearrange("c (s k) -> c s k", k=DE),
                P_rs[0][:, 0:3, :].transpose([1, 0, 2]))
            for c in range(3):
                nc.scalar.dma_start(
                    av6[3 + c:4 + c, :].rearrange("o (s k) -> o s k", k=DE),
                    P_rs[0][:, 3:4, :].transpose([1, 0, 2]))

            # scatter the per-sample diagonal blocks into bd_a / bd_v
            for s in range(SL):
                eng = nc.sync if s % 2 == 0 else nc.scalar
                sp = slice(3 * s, 3 * s + 3)
                kb = slice(DE * s, DE * (s + 1))
                eng.dma_start(bd_a[sp, kb], av48[sp, 0:128])
                eng.dma_start(bd_v[sp, kb], av48[sp, 128:256])

            # ---------- B = A @ av via block-diag kron(I16, A^T) ----------
            b48 = sb.tile([48, 512], BF16, tag="b48")
            with tc.tile_pool(name="bp_ps", bufs=1, space="PSUM") as bps:
                pb48 = bps.tile([48, 512], F32, tag="pb48")
                nc.tensor.matmul(pb48[:, 0:256], bdAa_t[:], av48[:],
                                 start=True, stop=True)
                nc.tensor.matmul(pb48[:, 256:512], bdAv_t[:], av48[:],
                                 start=True, stop=True)
                nc.vector.tensor_copy(b48[:], pb48[:])

            with (
                tc.tile_pool(name="att_ps", bufs=2, space="PSUM") as aps,
                tc.tile_pool(name="h_ps", bufs=1, space="PSUM") as hps,
                tc.tile_pool(name="o_ps", bufs=1, space="PSUM") as ops_,
            ):
                # ---------- att = tanh((enc^T @ B) / 16) ----------
                # per (branch, m-half): lhsT = dense B columns [48, 128],
                # rhs = block-diag enc [48, (s,k)] chunks; the zeros kill
                # cross-sample terms so 4 N=512 matmuls cover 16 samples
                att = {}
                bd = {"a": bd_a, "v": bd_v}
                for bi, br in enumerate(("a", "v")):
                    for half in (0, 1):
                        lhs_off = bi * 256 + half * 128
                        att_sb = sb.tile([128, SK], BF16,
                                         tag=f"att_{br}_{half}",
                                         name=f"att_{br}_{half}")
                        att[(br, half)] = att_sb
                        for q in range(4):
                            ck = slice(q * 512, (q + 1) * 512)
                            pa = aps.tile([128, 512], F32, tag="attp")
                            nc.tensor.matmul(
                                pa[:], b48[:, lhs_off:lhs_off + 128],
                                bd[br][:, ck], start=True, stop=True)
                            nc.scalar.activation(
                                att_sb[:, ck], pa[:], AF.Tanh, scale=0.0625)

                # ---------- H = relu(att @ Wc^T + enc^T @ W) ----------
                # audio rows 0:32, visual rows 32:64 of one packed psum
                ht = sb.tile([2 * DA, SK], BF16, tag="ht")
                for q in range(4):
                    ck = slice(q * 512, (q + 1) * 512)
                    ph = hps.tile([2 * DA, 512], F32, tag="ph")
                    nc.tensor.matmul(ph[0:DA, :], wa3_t[:], av6a[:, ck],
                                     start=True, stop=False)
                    nc.tensor.matmul(ph[0:DA, :], wca_lo[:],
                                     att[("a", 0)][:, ck],
                                     start=False, stop=False)
                    nc.tensor.matmul(ph[0:DA, :], wca_hi[:],
                                     att[("a", 1)][:, ck],
                                     start=False, stop=True)
                    nc.tensor.matmul(ph[DA:2 * DA, :], wv3_t[:], av6v[:, ck],
                                     start=True, stop=False)
                    nc.tensor.matmul(ph[DA:2 * DA, :], wcv_lo[:],
                                     att[("v", 0)][:, ck],
                                     start=False, stop=False)
                    nc.tensor.matmul(ph[DA:2 * DA, :], wcv_hi[:],
                                     att[("v", 1)][:, ck],
                                     start=False, stop=True)
                    nc.scalar.activation(ht[:, ck], ph[:], AF.Relu)

                # ---------- out = Wh @ H + enc ----------
                outsb = sb.tile([6, SK], F32, tag="outsb")
                for q in range(4):
                    ck = slice(q * 512, (q + 1) * 512)
                    po = ops_.tile([6, 512], F32, tag="po")
                    nc.tensor.matmul(po[:], wh6_t[:], ht[:, ck],
                                     start=True, stop=True)
                    nc.vector.tensor_add(outsb[:, ck], po[:], av6[:, ck])

            if DBG:
                d1 = sb.tile([48, 256], F32, tag="d1")
                nc.vector.tensor_copy(d1[:], av48[:])
                nc.gpsimd.dma_start(dbg_av48[:], d1[:])
                d2 = sb.tile([48, 512], F32, tag="d2")
                nc.vector.tensor_copy(d2[:], b48[:])
                nc.gpsimd.dma_start(dbg_b48[:], d2[:])
                d3 = sb.tile([48, SK], F32, tag="d3")
                nc.vector.tensor_copy(d3[:], bd_a[:])
                nc.gpsimd.dma_start(dbg_bda[:], d3[:])
                d4 = sb.tile([128, SK], F32, tag="d4")
                nc.vector.tensor_copy(d4[:], att[("a", 0)][:])
                nc.gpsimd.dma_start(dbg_atta0[:], d4[:])
                d5 = sb.tile([2 * DA, SK], F32, tag="d5")
                nc.vector.tensor_copy(d5[:], ht[:])
                nc.gpsimd.dma_start(dbg_ht[:], d5[:])
            nc.gpsimd.dma_start(
                out[:, :, 0:DE].transpose([1, 0, 2]),
                outsb[0:3, :].rearrange("c (s k) -> c s k", k=DE))
            nc.gpsimd.dma_start(
                out[:, :, DE:2 * DE].transpose([1, 0, 2]),
                outsb[3:6, :].rearrange("c (s k) -> c s k", k=DE))

    nc.compile()
    return nc


_NC_CACHE = None


def _get_nc():
    global _NC_CACHE
    if _NC_CACHE is None:
        _NC_CACHE = build_bass()
    return _NC_CACHE


def _prep_inputs(f1_norm, f2_norm, W1, b1, W2, b2, Aa, Av, Wa, Wv,
                 Wca, Wcv, Wha, Whv):
    f1_norm = np.asarray(f1_norm, dtype=np.float32)
    f2_norm = np.asarray(f2_norm, dtype=np.float32)
    bf = _np_dt(BF16)

    # nearest-downsample + transpose to [d, (c, s)] / [d, s]; pack with the
    # transposed weights into one contiguous per-core stream blob
    a_ds = f1_norm[:, :, ::4, ::4].reshape(B, 3, D)        # (B, 3, D)
    aT = a_ds.transpose(2, 1, 0).reshape(D, 3 * B)         # (D, 384)
    vT = f2_norm[:, ::4, ::4].reshape(B, D).T              # (D, 128)
    w1T = np.asarray(W1).T                                 # (D, 128)
    w2T = np.asarray(W2).T
    pack_full = np.concatenate([aT, vT, w1T, w2T], axis=1).astype(bf)
    pack_full = np.ascontiguousarray(pack_full)            # (D, 768)

    eye16 = np.eye(16, dtype=np.float32)
    wh6 = np.zeros((2 * DA, 6), dtype=np.float32)
    wh6[0:DA, 0:3] = np.asarray(Wha).T
    wh6[DA:2 * DA, 3:6] = np.asarray(Whv).T
    wh6 = wh6.astype(bf)
    consts = {
        "ident": np.eye(128, dtype=np.float32).astype(bf),
        "ones384": np.ones((1, 384), dtype=np.float32).astype(bf),
        "b1col": (np.asarray(b1)[None, :] / NC_).astype(bf),
        "b2col": (np.asarray(b2)[None, :] / NC_).astype(bf),
        "bdAaT": np.kron(eye16, np.asarray(Aa).T).astype(bf),
        "bdAvT": np.kron(eye16, np.asarray(Av).T).astype(bf),
        "wa3": np.ascontiguousarray(np.asarray(Wa).T).astype(bf),
        "wv3": np.ascontiguousarray(np.asarray(Wv).T).astype(bf),
        "wcaT": np.ascontiguousarray(np.asarray(Wca).T).astype(bf),
        "wcvT": np.ascontiguousarray(np.asarray(Wcv).T).astype(bf),
        "wh6": wh6,
    }

    in_maps = []
    for i in range(NC_):
        m = {"pack": np.ascontiguousarray(
            pack_full[i * DL:(i + 1) * DL]).reshape(NT, 128, 768)}
        m.update(consts)
        in_maps.append(m)
    return in_maps


def _run(inputs, trace=False):
    nc = _get_nc()
    in_maps = _prep_inputs(**inputs)
    res = run_bass_kernel_spmd(nc, in_maps, list(range(NC_)), trace=trace)
    full = np.concatenate([res.results[i]["out"] for i in range(NC_)], axis=0)
    return full.astype(np.float32, copy=False), res


def kernel(**inputs):
    out, _ = _run(inputs, trace=False)
    return out


# revision 19
# speedup vs baseline: 1.6060x; 1.3595x over previous
"""Trainium2 Bass kernel for nn_CrossAttentionModel (8 NeuronCores).

Strategy (v3): PURE batch-parallel, zero collectives.

Measured on this fabric: any collective costs ~10us of latency, the
auto-inserted prelude barrier another ~10us, they serialize on the cc
stream, and the first sync point absorbs the full cross-core NEFF start
skew (~35-40us) into the slowest-measured core.  A d-sharded encoder +
ReduceScatter therefore has a ~100us floor even though it moves 3.4x
fewer bytes.  Replicating the (small) encoder weights and giving each
core 16 whole samples runs collective-free: per-core cost is just its
own DMA stream (13.1 MB bf16 ~ 37us at 358 GB/s) overlapped with PE.

Encoder: per core, one contiguous p-major bf16 blob pack[128, 160*320]:
for each of 160 k-tiles (128 d-rows) the 320 cols are [a(s,i) 48 | v 16
| W1^T 128 | W2^T 128].  One DMA per 8 tiles.  Per tile ONE matmul:
stationary act[128,64], moving w12[128,256] -> psum[64,256] accumulates
aud rows (s,i)x cols 0:128 and vis rows 48:64 x cols 128:256 (the two
cross quadrants are garbage and unused).  Bias is seeded by a rank-1
ones x [b1|b2] matmul.

Attention (on-chip, no DRAM round-trip): rows stay in the [(s,i), k]
layout.  Block-diag kron(I16, A^T) stationaries make the 3x3 channel
mixes one matmul per branch-half; per-sample block-diag enc tiles
(bd_a/bd_v, 16 small SBUF DMAs each) turn the per-sample K=3 attention
maps into 4 dense K=48 matmuls per branch-half; tanh on ACT; H and out
projections are dense matmuls with the two branches packed at psum
partition bases 0/32.  The [3,(s,k)] gathers needed for the Wa/Wv terms
and the residual are extracted from bd_a/bd_v with a tile(I3) matmul
(block-diag zeros kill the cross-sample terms), because strided-
partition DMA *sources* are silently broken on this stack.
"""
import sys
sys.path.insert(0, "/opt/trn_rl_repo")

import numpy as np
import concourse.bass as bass
import concourse.mybir as mybir
import concourse.tile as tile
from concourse import bacc
from concourse.bass_utils import run_bass_kernel_spmd

F32 = mybir.dt.float32
BF16 = mybir.dt.bfloat16
AF = mybir.ActivationFunctionType

# ---- problem constants (hardcoded; kernel.py must be self-contained) ----
B, C, H, W = 128, 3, 512, 640
D = 20480            # (H//4) * (W//4)
DE = 128             # encoder dim
DA = 32              # attention dim
NC_ = 8              # cores
SL = B // NC_        # 16 samples per core
SK = SL * DE         # 2048 = (sample, enc) flattened cols
NT = D // 128        # 160 k-tiles (full contraction, per core)
TW = 64 + 2 * DE     # 320 cols per k-tile in the stream blob
CH = 8               # k-tiles per stream DMA
NCH = NT // CH       # 20 stream chunks


def _np_dt(dt):
    return mybir.dt.np(dt)


def build_bass():
    nc = bacc.Bacc("TRN2", target_bir_lowering=False, debug=False,
                   num_devices=NC_)

    # ---- per-core DRAM parameters ----
    pack = nc.declare_dram_parameter("pack", [128, NT * TW], BF16,
                                     isOutput=False)
    ones64 = nc.declare_dram_parameter("ones64", [1, 64], BF16,
                                       isOutput=False)
    brow = nc.declare_dram_parameter("brow", [1, 256], BF16, isOutput=False)
    bdAaT = nc.declare_dram_parameter("bdAaT", [48, 48], BF16, isOutput=False)
    bdAvT = nc.declare_dram_parameter("bdAvT", [48, 48], BF16, isOutput=False)
    tI3 = nc.declare_dram_parameter("tI3", [48, 3], BF16, isOutput=False)
    wa3 = nc.declare_dram_parameter("wa3", [3, DA], BF16, isOutput=False)
    wv3 = nc.declare_dram_parameter("wv3", [3, DA], BF16, isOutput=False)
    wcaT = nc.declare_dram_parameter("wcaT", [2 * DE, DA], BF16,
                                     isOutput=False)
    wcvT = nc.declare_dram_parameter("wcvT", [2 * DE, DA], BF16,
                                     isOutput=False)
    wh6 = nc.declare_dram_parameter("wh6", [2 * DA, 6], BF16, isOutput=False)
    out = nc.declare_dram_parameter("out", [SL, 3, 2 * DE], F32,
                                    isOutput=True)

    with tile.TileContext(nc) as tc:
        with (
            tc.tile_pool(name="consts", bufs=1) as cpool,
            tc.tile_pool(name="stream", bufs=3) as spool,
            tc.tile_pool(name="sb", bufs=1) as sb,
        ):
            # ---------- small consts (gpsimd queue) ----------
            ones_t = cpool.tile([1, 64], BF16)
            nc.gpsimd.dma_start(ones_t[:], ones64[:])
            brow_t = cpool.tile([1, 256], BF16)
            nc.gpsimd.dma_start(brow_t[:], brow[:])
            bdAa_t = cpool.tile([48, 48], BF16)
            nc.gpsimd.dma_start(bdAa_t[:], bdAaT[:])
            bdAv_t = cpool.tile([48, 48], BF16)
            nc.gpsimd.dma_start(bdAv_t[:], bdAvT[:])
            tI3_t = cpool.tile([48, 3], BF16)
            nc.gpsimd.dma_start(tI3_t[:], tI3[:])
            wa3_t = cpool.tile([3, DA], BF16)
            nc.gpsimd.dma_start(wa3_t[:], wa3[:])
            wv3_t = cpool.tile([3, DA], BF16)
            nc.gpsimd.dma_start(wv3_t[:], wv3[:])
            wca_lo = cpool.tile([DE, DA], BF16)
            nc.gpsimd.dma_start(wca_lo[:], wcaT[0:DE, :])
            wca_hi = cpool.tile([DE, DA], BF16)
            nc.gpsimd.dma_start(wca_hi[:], wcaT[DE:2 * DE, :])
            wcv_lo = cpool.tile([DE, DA], BF16)
            nc.gpsimd.dma_start(wcv_lo[:], wcvT[0:DE, :])
            wcv_hi = cpool.tile([DE, DA], BF16)
            nc.gpsimd.dma_start(wcv_hi[:], wcvT[DE:2 * DE, :])
            wh6_t = cpool.tile([2 * DA, 6], BF16)
            nc.gpsimd.dma_start(wh6_t[:], wh6[:])

            # block-diag enc tiles, zeroed early (runs during the encoder)
            bd_a = sb.tile([48, SK], BF16, tag="bd_a")
            nc.vector.memset(bd_a[:], 0.0)
            bd_v = sb.tile([48, SK], BF16, tag="bd_v")
            nc.gpsimd.memset(bd_v[:], 0.0)

            # ---------- phase 1: encoder (all 160 k-tiles, one psum) -----
            enc_sb = sb.tile([64, 256], BF16, tag="enc_sb")
            with tc.tile_pool(name="enc_ps", bufs=1, space="PSUM") as eps:
                psum = eps.tile([64, 256], F32, tag="enc")
                nc.tensor.matmul(psum[:], ones_t[:], brow_t[:],
                                 start=True, stop=False)
                for ch in range(NCH):
                    st = spool.tile([128, CH * TW], BF16, tag="st")
                    nc.sync.dma_start(
                        st[:], pack[:, ch * CH * TW:(ch + 1) * CH * TW])
                    for j in range(CH):
                        o = j * TW
                        last = ch == NCH - 1 and j == CH - 1
                        nc.tensor.matmul(psum[:], st[:, o:o + 64],
                                         st[:, o + 64:o + TW],
                                         start=False, stop=last)
                nc.vector.tensor_copy(enc_sb[:], psum[:])

            # ---------- phase 2: attention (on-chip, 16 samples) ---------
            # vis replicated x3 into (s,i) rows  (dst-strided DMA is fine)
            visrep = sb.tile([48, 128], BF16, tag="visrep")
            v3v = visrep[:].rearrange("(s i) k -> s i k", i=3)
            for i in range(3):
                nc.scalar.dma_start(v3v[:, i, :], enc_sb[48:64, 128:256])

            # per-sample block-diag fills (spread over 4 queues)
            engs = [nc.sync, nc.gpsimd, nc.scalar]
            for s in range(SL):
                sp = slice(3 * s, 3 * s + 3)
                kb = slice(DE * s, DE * (s + 1))
                engs[s % 3].dma_start(bd_a[sp, kb], enc_sb[sp, 0:128])
                engs[(s + 1) % 3].dma_start(bd_v[sp, kb], visrep[sp, :])

            # B = A @ av via block-diag kron(I16, A^T):  b48 [48, 512] =
            # [Aa@aud | Aa@vis | Av@aud | Av@vis] in m-halves of 128
            b48 = sb.tile([48, 512], BF16, tag="b48")
            av6 = sb.tile([6, SK], BF16, tag="av6")
            av6a = sb.tile([3, SK], BF16, tag="av6a")
            av6v = sb.tile([3, SK], BF16, tag="av6v")
            with (
                tc.tile_pool(name="bp_ps", bufs=1, space="PSUM") as bps,
                tc.tile_pool(name="g3_ps", bufs=2, space="PSUM") as gps,
            ):
                pb48 = bps.tile([48, 512], F32, tag="pb48")
                aud_rhs = enc_sb[0:48, 0:128]
                nc.tensor.matmul(pb48[:, 0:128], bdAa_t[:], aud_rhs,
                                 start=True, stop=True)
                nc.tensor.matmul(pb48[:, 128:256], bdAa_t[:], visrep[:],
                                 start=True, stop=True)
                nc.tensor.matmul(pb48[:, 256:384], bdAv_t[:], aud_rhs,
                                 start=True, stop=True)
                nc.tensor.matmul(pb48[:, 384:512], bdAv_t[:], visrep[:],
                                 start=True, stop=True)
                nc.vector.tensor_copy(b48[:], pb48[:])

                # [3,(s,k)] gathers via tile(I3) against the block-diags
                for q in range(4):
                    ck = slice(q * 512, (q + 1) * 512)
                    ga = gps.tile([3, 512], F32, tag="ga")
                    nc.tensor.matmul(ga[:], tI3_t[:], bd_a[:, ck],
                                     start=True, stop=True)
                    nc.vector.tensor_copy(av6a[:, ck], ga[:])
                    gv = gps.tile([3, 512], F32, tag="gv")
                    nc.tensor.matmul(gv[:], tI3_t[:], bd_v[:, ck],
                                     start=True, stop=True)
                    nc.vector.tensor_copy(av6v[:, ck], gv[:])
            nc.gpsimd.dma_start(av6[0:3, :], av6a[:])
            nc.gpsimd.dma_start(av6[3:6, :], av6v[:])

            with (
                tc.tile_pool(name="att_ps", bufs=2, space="PSUM") as aps,
                tc.tile_pool(name="h_ps", bufs=2, space="PSUM") as hps,
                tc.tile_pool(name="o_ps", bufs=2, space="PSUM") as ops_,
            ):
                # ---------- att = tanh((enc^T @ B) / 16) ----------
                att = {}
                bd = {"a": bd_a, "v": bd_v}
                for bi, br in enumerate(("a", "v")):
                    for half in (0, 1):
                        lhs_off = bi * 256 + half * 128
                        att_sb = sb.tile([128, SK], BF16,
                                         tag=f"att_{br}_{half}",
                                         name=f"att_{br}_{half}")
                        att[(br, half)] = att_sb
                        for q in range(4):
                            ck = slice(q * 512, (q + 1) * 512)
                            pa = aps.tile([128, 512], F32, tag="attp")
                            nc.tensor.matmul(
                                pa[:], b48[:, lhs_off:lhs_off + 128],
                                bd[br][:, ck], start=True, stop=True)
                            nc.scalar.activation(
                                att_sb[:, ck], pa[:], AF.Tanh, scale=0.0625)

                # ---------- H = relu(att @ Wc^T + enc^T @ W) ----------
                # audio rows 0:32, visual rows 32:64 of one packed psum
                ht = sb.tile([2 * DA, SK], BF16, tag="ht")
                for q in range(4):
                    ck = slice(q * 512, (q + 1) * 512)
                    ph = hps.tile([2 * DA, 512], F32, tag="ph")
                    nc.tensor.matmul(ph[0:DA, :], wa3_t[:], av6a[:, ck],
                                     start=True, stop=False)
                    nc.tensor.matmul(ph[0:DA, :], wca_lo[:],
                                     att[("a", 0)][:, ck],
                                     start=False, stop=False)
                    nc.tensor.matmul(ph[0:DA, :], wca_hi[:],
                                     att[("a", 1)][:, ck],
                                     start=False, stop=True)
                    nc.tensor.matmul(ph[DA:2 * DA, :], wv3_t[:],
                                     av6v[:, ck], start=True, stop=False)
                    nc.tensor.matmul(ph[DA:2 * DA, :], wcv_lo[:],
                                     att[("v", 0)][:, ck],
                                     start=False, stop=False)
                    nc.tensor.matmul(ph[DA:2 * DA, :], wcv_hi[:],
                                     att[("v", 1)][:, ck],
                                     start=False, stop=True)
                    nc.vector.tensor_relu(ht[:, ck], ph[:])

                # ---------- out = Wh @ H + enc ----------
                outsb = sb.tile([6, SK], F32, tag="outsb")
                for q in range(4):
                    ck = slice(q * 512, (q + 1) * 512)
                    po = ops_.tile([6, 512], F32, tag="po")
                    nc.tensor.matmul(po[:], wh6_t[:], ht[:, ck],
                                     start=True, stop=True)
                    nc.vector.tensor_add(outsb[:, ck], po[:], av6[:, ck])

            nc.gpsimd.dma_start(
                out[:, :, 0:DE].transpose([1, 0, 2]),
                outsb[0:3, :].rearrange("c (s k) -> c s k", k=DE))
            nc.gpsimd.dma_start(
                out[:, :, DE:2 * DE].transpose([1, 0, 2]),
                outsb[3:6, :].rearrange("c (s k) -> c s k", k=DE))

    nc.compile()
    return nc


_NC_CACHE = None


def _get_nc():
    global _NC_CACHE
    if _NC_CACHE is None:
        _NC_CACHE = build_bass()
    return _NC_CACHE


def _prep_inputs(f1_norm, f2_norm, W1, b1, W2, b2, Aa, Av, Wa, Wv,
                 Wca, Wcv, Wha, Whv):
    f1_norm = np.asarray(f1_norm, dtype=np.float32)
    f2_norm = np.asarray(f2_norm, dtype=np.float32)
    bf = _np_dt(BF16)

    a_ds = f1_norm[:, :, ::4, ::4].reshape(B, 3, D)        # (B, 3, D)
    v_ds = f2_norm[:, ::4, ::4].reshape(B, D)
    w1T = np.asarray(W1).T.astype(bf)                      # (D, 128)
    w2T = np.asarray(W2).T.astype(bf)

    eye16 = np.eye(16, dtype=np.float32)
    wh6_np = np.zeros((2 * DA, 6), dtype=np.float32)
    wh6_np[0:DA, 0:3] = np.asarray(Wha).T
    wh6_np[DA:2 * DA, 3:6] = np.asarray(Whv).T
    consts = {
        "ones64": np.ones((1, 64), dtype=np.float32).astype(bf),
        "brow": np.concatenate([np.asarray(b1), np.asarray(b2)])[None, :]
        .astype(bf),
        "bdAaT": np.kron(eye16, np.asarray(Aa).T).astype(bf),
        "bdAvT": np.kron(eye16, np.asarray(Av).T).astype(bf),
        "tI3": np.tile(np.eye(3, dtype=np.float32), (16, 1)).astype(bf),
        "wa3": np.ascontiguousarray(np.asarray(Wa).T).astype(bf),
        "wv3": np.ascontiguousarray(np.asarray(Wv).T).astype(bf),
        "wcaT": np.ascontiguousarray(np.asarray(Wca).T).astype(bf),
        "wcvT": np.ascontiguousarray(np.asarray(Wcv).T).astype(bf),
        "wh6": wh6_np.astype(bf),
    }

    in_maps = []
    for i in range(NC_):
        sl = slice(i * SL, (i + 1) * SL)
        # stream[d, 0:48] = a[(s,i)], [48:64] = v[s], then W1^T | W2^T
        aT48 = a_ds[sl].transpose(2, 0, 1).reshape(D, 48).astype(bf)
        vT16 = v_ds[sl].T.astype(bf)
        stream = np.concatenate([aT48, vT16, w1T, w2T], axis=1)  # (D, 320)
        blob = np.ascontiguousarray(
            stream.reshape(NT, 128, TW).transpose(1, 0, 2)
        ).reshape(128, NT * TW)
        m = {"pack": blob}
        m.update(consts)
        in_maps.append(m)
    return in_maps


def _run(inputs, trace=False):
    nc = _get_nc()
    in_maps = _prep_inputs(**inputs)
    res = run_bass_kernel_spmd(nc, in_maps, list(range(NC_)), trace=trace)
    full = np.concatenate([res.results[i]["out"] for i in range(NC_)], axis=0)
    return full.astype(np.float32, copy=False), res


def kernel(**inputs):
    out, _ = _run(inputs, trace=False)
    return out
